# revision 1
# baseline (speedup 1.0000x reference)
"""ConvGRU Trainium2 Bass kernel.

Math: ConvGRU cell with 3 gates (z, r, q), each gate = depthwise 3x3 conv
(SAME) followed by pointwise 1x1 conv, weights int8-fake-quantized
per-tensor.

Strategy:
  - Data-parallel over batch: 8 images -> 8 NeuronCores, one image each.
  - The depthwise+pointwise composition is folded into 9 shifted matmuls
    accumulated in PSUM:  p = sum_t (Wp . diag(Wd_t)) @ shift_t(hx).
    Shifts are free-dim AP offsets into a zero-padded SBUF image
    (W 128->132, H 64->66).
  - Weights are factored: int8 integer parts (exact in bf16) go into the
    matmuls; per-tensor scales and the combined bias (Wp@bd + bp) are
    applied by the ScalarEngine fused into sigmoid/tanh.
  - Per-core image is processed in 22 row-windows (3 padded rows = 396
    cols per matmul, PSUM-bank sized).
"""

import sys

sys.path.insert(0, "/opt/trn_rl_repo")

import ml_dtypes
import numpy as np

HID, INP, C = 128, 320, 448
B, H, W = 8, 64, 128
Wp, Hp = 132, 66
PIX = Hp * Wp  # 8712
NPIX = H * W  # 8192
ROWS_PER_WIN = 3
NWIN = 22  # 21 windows x 3 rows + 1 window x 1 row

# channel chunks on partitions: [0:128)=h/rh, [128:256)=x0, [256:384)=x1, [384:448)=x2
CHUNKS = [(0, 128), (128, 256), (256, 384), (384, 448)]

_CACHE = {}


def _win_geom(w):
    rows = ROWS_PER_WIN if w < NWIN - 1 else H - ROWS_PER_WIN * (NWIN - 1)
    q0 = (1 + ROWS_PER_WIN * w) * Wp
    return q0, rows, rows * Wp


def _build(loop_reps=None):
    """Build the SPMD kernel. loop_reps wraps the whole body in an on-device
    For_i loop (identical code size for any trip count) — used by test.py to
    measure device time as a wall-clock slope between two trip counts."""
    import contextlib

    import concourse.bacc as bacc
    import concourse.tile as tile
    from concourse import mybir

    f32 = mybir.dt.float32
    bf16 = mybir.dt.bfloat16
    AF = mybir.ActivationFunctionType

    nc = bacc.Bacc("TRN2", target_bir_lowering=False, debug=False, num_devices=8)

    h32 = nc.dram_tensor("h32", [HID, NPIX], f32, kind="ExternalInput")
    x32 = nc.dram_tensor("x32", [INP, NPIX], f32, kind="ExternalInput")
    wz = nc.dram_tensor("wz", [9, C, HID], bf16, kind="ExternalInput")
    wr = nc.dram_tensor("wr", [9, C, HID], bf16, kind="ExternalInput")
    wq = nc.dram_tensor("wq", [9, C, HID], bf16, kind="ExternalInput")
    sbt_d = nc.dram_tensor("sbt", [HID, 6], f32, kind="ExternalInput")
    out_d = nc.dram_tensor("out", [HID, NPIX], f32, kind="ExternalOutput")

    with tile.TileContext(nc) as tc:
        with (
            tc.tile_pool(name="big", bufs=1) as big,
            tc.tile_pool(name="wp", bufs=1) as wpool,
            tc.tile_pool(name="stage", bufs=2) as stage,
            tc.tile_pool(name="win", bufs=3) as win,
            tc.tile_pool(name="psum", bufs=2, space="PSUM") as psum,
            tc.For_i(0, loop_reps, 1) if loop_reps else contextlib.nullcontext(),
        ):
            hpad = big.tile([128, PIX], bf16)
            xc0 = big.tile([128, PIX], bf16)
            xc1 = big.tile([128, PIX], bf16)
            xc2 = big.tile([64, PIX], bf16)
            rhpad = big.tile([128, PIX], bf16)
            zpad = big.tile([128, PIX], bf16)

            for t_ in (hpad, xc0, xc1, xc2, rhpad):
                nc.vector.memset(t_[:], 0.0)

            sbt = wpool.tile([128, 6], f32)
            nc.sync.dma_start(out=sbt[:], in_=sbt_d[:])

            # weight tiles: [gate][tap][chunk]
            wd = {"z": wz, "r": wr, "q": wq}
            wt = {}
            for g in ("z", "r", "q"):
                for t in range(9):
                    for ci, (c0, c1) in enumerate(CHUNKS):
                        wtile = wpool.tile(
                            [c1 - c0, 128], bf16, name=f"w_{g}_{t}_{ci}"
                        )
                        nc.sync.dma_start(out=wtile[:], in_=wd[g][t, c0:c1, :])
                        wt[(g, t, ci)] = wtile

            # load inputs + convert fp32 -> bf16 into padded layouts
            srcs = [hpad, xc0, xc1, xc2]
            for half in range(2):
                for ci, (c0, c1) in enumerate(CHUNKS):
                    kc = c1 - c0
                    dst3 = srcs[ci].rearrange("p (r c) -> p r c", c=Wp)
                    st = stage.tile([128, NPIX // 2], f32, tag="st",
                                    name=f"st{half}{ci}")
                    srcd = h32 if ci == 0 else x32
                    off = 0 if ci == 0 else c0 - 128
                    nc.sync.dma_start(
                        out=st[:kc, :],
                        in_=srcd[
                            off : off + kc,
                            half * (NPIX // 2) : (half + 1) * (NPIX // 2),
                        ],
                    )
                    st3 = st.rearrange("p (r c) -> p r c", c=W)
                    nc.vector.tensor_copy(
                        dst3[:kc, 1 + 32 * half : 33 + 32 * half, 1:129],
                        st3[:kc, :32, :],
                    )

            # center tap first: full-width matmul initializes every PSUM
            # column of the accumulation group (start=True); shifted taps
            # may be clamped at the image boundary.
            taps = sorted(
                [(ky - 1, kx - 1, 3 * ky + kx) for ky in range(3) for kx in range(3)],
                key=lambda t: (t[0] != 0 or t[1] != 0),
            )
            zr_srcs = [hpad, xc0, xc1, xc2]
            q_srcs = [rhpad, xc0, xc1, xc2]

            # ---- phase 1: z and r gates; build rh = r*h and store z ----
            for w in range(NWIN):
                q0, rows, n = _win_geom(w)
                pz = psum.tile([128, n], f32, tag="pz", name=f"pz{w}")
                pr = psum.tile([128, n], f32, tag="pr", name=f"pr{w}")
                i = 0
                for dy, dx, t in taps:
                    o = q0 + dy * Wp + dx
                    s, e = max(o, 0), min(o + n, PIX)
                    for ci, (c0, c1) in enumerate(CHUNKS):
                        kc = c1 - c0
                        rhs = zr_srcs[ci][:kc, s:e]
                        nc.tensor.matmul(
                            pz[:, s - o : s - o + (e - s)], wt[("z", t, ci)][:],
                            rhs, start=(i == 0), stop=(i == 35),
                        )
                        nc.tensor.matmul(
                            pr[:, s - o : s - o + (e - s)], wt[("r", t, ci)][:],
                            rhs, start=(i == 0), stop=(i == 35),
                        )
                        i += 1
                rwin = win.tile([128, n], bf16, tag="rwin", name=f"rw{w}")
                nc.scalar.activation(
                    rwin[:], pr[:], AF.Sigmoid, bias=sbt[:, 3:4], scale=sbt[:, 2:3]
                )
                nc.scalar.activation(
                    zpad[:, q0 : q0 + n], pz[:], AF.Sigmoid,
                    bias=sbt[:, 1:2], scale=sbt[:, 0:1],
                )
                nc.vector.tensor_mul(
                    rhpad[:, q0 : q0 + n], rwin[:], hpad[:, q0 : q0 + n]
                )

            # ---- phase 2: q gate + GRU mix ----
            out3 = out_d.rearrange("p (r c) -> p r c", c=W)
            for w in range(NWIN):
                q0, rows, n = _win_geom(w)
                pq = psum.tile([128, n], f32, tag="pq", name=f"pq{w}")
                i = 0
                for dy, dx, t in taps:
                    o = q0 + dy * Wp + dx
                    s, e = max(o, 0), min(o + n, PIX)
                    for ci, (c0, c1) in enumerate(CHUNKS):
                        kc = c1 - c0
                        rhs = q_srcs[ci][:kc, s:e]
                        nc.tensor.matmul(
                            pq[:, s - o : s - o + (e - s)], wt[("q", t, ci)][:],
                            rhs, start=(i == 0), stop=(i == 35),
                        )
                        i += 1
                qwin = win.tile([128, n], bf16, tag="qwin", name=f"qw{w}")
                nc.scalar.activation(
                    qwin[:], pq[:], AF.Tanh, bias=sbt[:, 5:6], scale=sbt[:, 4:5]
                )
                dwin = win.tile([128, n], bf16, tag="dwin", name=f"dw{w}")
                nc.vector.tensor_sub(dwin[:], qwin[:], hpad[:, q0 : q0 + n])
                mwin = win.tile([128, n], bf16, tag="mwin", name=f"mw{w}")
                nc.vector.tensor_mul(mwin[:], zpad[:, q0 : q0 + n], dwin[:])
                owin = win.tile([128, n], f32, tag="owin", name=f"ow{w}")
                nc.vector.tensor_add(owin[:], hpad[:, q0 : q0 + n], mwin[:])
                o3 = owin.rearrange("p (r c) -> p r c", c=Wp)
                y0 = ROWS_PER_WIN * w
                nc.sync.dma_start(
                    out=out3[:, y0 : y0 + rows, :], in_=o3[:, :rows, 1:129]
                )

    nc.compile()
    return nc


def _fq_int(w):
    w = np.asarray(w, np.float32)
    scale = (
        np.maximum(np.max(np.abs(w)), np.float32(1e-8)) / np.float32(127.0)
    ).astype(np.float32)
    q = np.clip(np.round(w / scale), -128, 127).astype(np.float32)
    return q, scale


def _prep_gate(wdg, bdg, wpg, bpg):
    qd, sd = _fq_int(wdg)  # [C,1,3,3]
    qp, sp = _fq_int(wpg)  # [HID,C,1,1]
    qp2 = qp[:, :, 0, 0]  # [HID, C]
    lhsT = np.empty((9, C, HID), np.float32)
    for ky in range(3):
        for kx in range(3):
            m = qp2 * qd[:, 0, ky, kx][None, :]  # [HID, C]
            lhsT[3 * ky + kx] = m.T
    scale = np.float32(sd) * np.float32(sp)
    bias = (
        np.float32(sp) * (qp2 @ np.asarray(bdg, np.float32))
        + np.asarray(bpg, np.float32)
    ).astype(np.float32)
    return lhsT.astype(ml_dtypes.bfloat16), scale, bias


def last_in_maps(inputs):
    h = np.asarray(inputs["h"], np.float32)
    x = np.asarray(inputs["x"], np.float32)

    wz, s_z, b_z = _prep_gate(
        inputs["wdz"], inputs["bdz"], inputs["wpz"], inputs["bpz"]
    )
    wr, s_r, b_r = _prep_gate(
        inputs["wdr"], inputs["bdr"], inputs["wpr"], inputs["bpr"]
    )
    wq, s_q, b_q = _prep_gate(
        inputs["wdq"], inputs["bdq"], inputs["wpq"], inputs["bpq"]
    )

    sbt = np.empty((HID, 6), np.float32)
    sbt[:, 0] = s_z
    sbt[:, 1] = b_z
    sbt[:, 2] = s_r
    sbt[:, 3] = b_r
    sbt[:, 4] = s_q
    sbt[:, 5] = b_q

    in_maps = []
    for i in range(B):
        in_maps.append(
            {
                "h32": np.ascontiguousarray(h[i].reshape(HID, NPIX)),
                "x32": np.ascontiguousarray(x[i].reshape(INP, NPIX)),
                "wz": wz,
                "wr": wr,
                "wq": wq,
                "sbt": sbt,
            }
        )
    return in_maps


def kernel(**inputs):
    from concourse.bass_utils import run_bass_kernel_spmd

    if "nc" not in _CACHE:
        _CACHE["nc"] = _build()
    nc = _CACHE["nc"]

    in_maps = last_in_maps(inputs)

    res = run_bass_kernel_spmd(nc, in_maps, list(range(B)))
    out = np.stack(
        [res.results[i]["out"].reshape(HID, H, W) for i in range(B)], axis=0
    )
    return out.astype(np.float32)



# revision 2
# speedup vs baseline: 20.3552x; 20.3552x over previous
"""ConvGRU Trainium2 Bass kernel (fp8 DoubleRow edition).

Math: ConvGRU cell with 3 gates (z, r, q), each gate = depthwise 3x3 conv
(SAME) followed by pointwise 1x1 conv, weights int8-fake-quantized
per-tensor.

Strategy:
  - Data-parallel over batch: 8 images -> 8 NeuronCores, one image each.
  - The depthwise+pointwise composition is folded into 9 shifted matmuls
    accumulated in PSUM:  p = sum_t (Wp . diag(Wd_t)) @ shift_t(hx).
    Shifts are free-dim AP offsets into a zero-padded SBUF image
    (W 128->132, H 64->66).
  - fp8e4m3 + MatmulPerfMode.DoubleRow: each PE cell holds 2 weights and
    contracts 2 channel-planes per pass, so the 448-channel contraction
    takes 2 matmuls per tap (a 128x2 plane-pair and a 96x2 plane-pair)
    instead of 4 bf16 ones.  Channel plane pairing:
      AB  tile [128, 2, PIX]: plane0 = h (or r*h), plane1 = x[0:128]
      CD  tile [ 96, 2, PIX]: plane0 = x[128:224],
                              plane1 = x[224:256] (parts 0-31)
                                     | x[256:320] (parts 32-95)
  - Folded weights are scaled by 1/128 into fp8 range; the scale is
    folded back in the ScalarEngine activation (sigmoid/tanh) together
    with the combined bias (Wp@bd + bp), applied in fp32.
  - Per-core image processed in 22 row-windows (3 padded rows = 396 cols
    per matmul, PSUM-bank sized).  GRU mix runs in fp32 on DVE.
"""

import sys

sys.path.insert(0, "/opt/trn_rl_repo")

import ml_dtypes
import numpy as np

HID, INP, C = 128, 320, 448
B, H, W = 8, 64, 128
Wp, Hp = 132, 66
PIX = Hp * Wp  # 8712
NPIX = H * W  # 8192
ROWS_PER_WIN = 3
NWIN = 22  # 21 windows x 3 rows + 1 window x 1 row
NQ = 4  # input staged in quarters of 16 image rows
QROWS = H // NQ
QPIX = QROWS * W  # 2048
DIV = np.float32(128.0)  # fp8 range scaling for folded weights

FP8 = ml_dtypes.float8_e4m3

_CACHE = {}


def _win_geom(w):
    rows = ROWS_PER_WIN if w < NWIN - 1 else H - ROWS_PER_WIN * (NWIN - 1)
    q0 = (1 + ROWS_PER_WIN * w) * Wp
    return q0, rows, rows * Wp


def _build(loop_reps=None):
    """Build the SPMD kernel. loop_reps wraps the whole body in an on-device
    For_i loop (identical code size for any trip count) — used by test.py to
    measure device time as a wall-clock slope between two trip counts."""
    import contextlib

    import concourse.bacc as bacc
    import concourse.tile as tile
    from concourse import mybir

    f32 = mybir.dt.float32
    bf16 = mybir.dt.bfloat16
    fp8 = mybir.dt.float8e4
    AF = mybir.ActivationFunctionType
    DR = mybir.MatmulPerfMode.DoubleRow

    nc = bacc.Bacc("TRN2", target_bir_lowering=False, debug=False, num_devices=8)

    h32 = nc.dram_tensor("h32", [HID, NPIX], f32, kind="ExternalInput")
    x32 = nc.dram_tensor("x32", [INP, NPIX], f32, kind="ExternalInput")
    wab = {
        g: nc.dram_tensor(f"wab{g}", [128, 9 * 256], fp8, kind="ExternalInput")
        for g in ("z", "r", "q")
    }
    wcd = {
        g: nc.dram_tensor(f"wcd{g}", [96, 9 * 256], fp8, kind="ExternalInput")
        for g in ("z", "r", "q")
    }
    sbt_d = nc.dram_tensor("sbt", [HID, 6], f32, kind="ExternalInput")
    out_d = nc.dram_tensor("out", [HID, NPIX], f32, kind="ExternalOutput")

    with tile.TileContext(nc) as tc:
        with (
            tc.tile_pool(name="big", bufs=1) as big,
            tc.tile_pool(name="wp", bufs=1) as wpool,
            tc.tile_pool(name="stage", bufs=3) as stage,
            tc.tile_pool(name="win", bufs=2) as win,
            tc.tile_pool(name="psum", bufs=2, space="PSUM") as psum,
            tc.For_i(0, loop_reps, 1) if loop_reps else contextlib.nullcontext(),
        ):
            ab = big.tile([128, 2 * PIX], fp8)  # plane0 h, plane1 x0
            apb = big.tile([128, 2 * PIX], fp8)  # plane0 r*h, plane1 x0
            cd = big.tile([96, 2 * PIX], fp8)  # x1 | x1-tail+x2
            hpad = big.tile([128, PIX], bf16)
            zpad = big.tile([128, PIX], f32)

            ab4 = ab.rearrange("p (j r c) -> p j r c", j=2, c=Wp)
            apb4 = apb.rearrange("p (j r c) -> p j r c", j=2, c=Wp)
            cd4 = cd.rearrange("p (j r c) -> p j r c", j=2, c=Wp)
            hp3 = hpad.rearrange("p (r c) -> p r c", c=Wp)

            # zero the halo borders (rows 0/65, cols 0 and 129-131)
            for t4 in (ab4, apb4, cd4):
                nc.vector.memset(t4[:, :, 0, :], 0.0)
                nc.vector.memset(t4[:, :, Hp - 1, :], 0.0)
                nc.vector.memset(t4[:, :, 1 : Hp - 1, 0:1], 0.0)
                nc.vector.memset(t4[:, :, 1 : Hp - 1, W + 1 :], 0.0)
            nc.vector.memset(hp3[:, 0, :], 0.0)
            nc.vector.memset(hp3[:, Hp - 1, :], 0.0)
            nc.vector.memset(hp3[:, 1 : Hp - 1, 0:1], 0.0)
            nc.vector.memset(hp3[:, 1 : Hp - 1, W + 1 :], 0.0)

            sbt = wpool.tile([128, 6], f32)
            nc.sync.dma_start(out=sbt[:], in_=sbt_d[:])

            wt = {}
            for g in ("z", "r", "q"):
                tab = wpool.tile([128, 9 * 256], fp8, name=f"wab_{g}")
                nc.sync.dma_start(out=tab[:], in_=wab[g][:])
                tcd = wpool.tile([96, 9 * 256], fp8, name=f"wcd_{g}")
                nc.sync.dma_start(out=tcd[:], in_=wcd[g][:])
                wt[g] = (
                    tab.rearrange("p (t j m) -> p t j m", t=9, j=2),
                    tcd.rearrange("p (t j m) -> p t j m", t=9, j=2),
                )

            # load inputs in quarters; convert fp32 -> bf16/fp8 padded layouts
            for q in range(NQ):
                r0 = q * QROWS  # image row of this quarter
                csl = slice(q * QPIX, (q + 1) * QPIX)

                sth = stage.tile([128, QPIX], f32, tag="st", name=f"sth{q}")
                nc.sync.dma_start(out=sth[:], in_=h32[:, csl])
                sth3 = sth.rearrange("p (r c) -> p r c", c=W)
                dr = slice(1 + r0, 1 + r0 + QROWS)
                nc.vector.tensor_copy(hp3[:, dr, 1 : W + 1], sth3[:])
                nc.vector.tensor_copy(ab4[:, 0, dr, 1 : W + 1], sth3[:])

                stx = stage.tile([128, QPIX], f32, tag="st", name=f"stx{q}")
                nc.sync.dma_start(out=stx[:], in_=x32[0:128, csl])
                stx3 = stx.rearrange("p (r c) -> p r c", c=W)
                nc.vector.tensor_copy(ab4[:, 1, dr, 1 : W + 1], stx3[:])
                nc.scalar.activation(apb4[:, 1, dr, 1 : W + 1], stx3[:], AF.Copy)

                stc = stage.tile([96, QPIX], f32, tag="st", name=f"stc{q}")
                nc.sync.dma_start(out=stc[:], in_=x32[128:224, csl])
                stc3 = stc.rearrange("p (r c) -> p r c", c=W)
                nc.vector.tensor_copy(cd4[:, 0, dr, 1 : W + 1], stc3[:])

                std = stage.tile([96, QPIX], f32, tag="st", name=f"std{q}")
                nc.sync.dma_start(out=std[0:32, :], in_=x32[224:256, csl])
                nc.sync.dma_start(out=std[32:96, :], in_=x32[256:320, csl])
                std3 = std.rearrange("p (r c) -> p r c", c=W)
                nc.scalar.activation(cd4[:, 1, dr, 1 : W + 1], std3[:], AF.Copy)

            # center tap first: full-width matmul initializes every PSUM
            # column of the accumulation group (start=True); shifted taps
            # may be clamped at the image boundary.
            taps = sorted(
                [(ky - 1, kx - 1, 3 * ky + kx) for ky in range(3) for kx in range(3)],
                key=lambda t: (t[0] != 0 or t[1] != 0),
            )

            ab2 = ab.rearrange("p (j q) -> p j q", j=2)
            apb2 = apb.rearrange("p (j q) -> p j q", j=2)
            cd2 = cd.rearrange("p (j q) -> p j q", j=2)

            def gate_mms(pt, g, rhs_ab, w):
                q0, rows, n = _win_geom(w)
                tab, tcd = wt[g]
                i = 0
                for dy, dx, t in taps:
                    o = q0 + dy * Wp + dx
                    s, e = max(o, 0), min(o + n, PIX)
                    d = slice(s - o, s - o + (e - s))
                    nc.tensor.matmul(
                        pt[:, d], tab[:, t], rhs_ab[:, :, s:e],
                        start=(i == 0), stop=False, perf_mode=DR,
                    )
                    nc.tensor.matmul(
                        pt[:, d], tcd[:, t], cd2[:, :, s:e],
                        start=False, stop=(i == 8), perf_mode=DR,
                    )
                    i += 1

            # ---- phase 1: z and r gates; build A'B' plane0 = r*h, store z ----
            for w in range(NWIN):
                q0, rows, n = _win_geom(w)
                pz = psum.tile([128, n], f32, tag="pz", name=f"pz{w}")
                pr = psum.tile([128, n], f32, tag="pr", name=f"pr{w}")
                gate_mms(pz, "z", ab2, w)
                gate_mms(pr, "r", ab2, w)
                rwin = win.tile([128, n], f32, tag="rwin", name=f"rw{w}")
                nc.scalar.activation(
                    rwin[:], pr[:], AF.Sigmoid, bias=sbt[:, 3:4], scale=sbt[:, 2:3]
                )
                nc.scalar.activation(
                    zpad[:, q0 : q0 + n], pz[:], AF.Sigmoid,
                    bias=sbt[:, 1:2], scale=sbt[:, 0:1],
                )
                nc.vector.tensor_mul(
                    apb2[:, 0, q0 : q0 + n], rwin[:], hpad[:, q0 : q0 + n]
                )

            # ---- phase 2: q gate + GRU mix ----
            out3 = out_d.rearrange("p (r c) -> p r c", c=W)
            for w in range(NWIN):
                q0, rows, n = _win_geom(w)
                pq = psum.tile([128, n], f32, tag="pq", name=f"pq{w}")
                gate_mms(pq, "q", apb2, w)
                qwin = win.tile([128, n], f32, tag="qwin", name=f"qw{w}")
                nc.scalar.activation(
                    qwin[:], pq[:], AF.Tanh, bias=sbt[:, 5:6], scale=sbt[:, 4:5]
                )
                dwin = win.tile([128, n], f32, tag="dwin", name=f"dw{w}")
                nc.vector.tensor_sub(dwin[:], qwin[:], hpad[:, q0 : q0 + n])
                mwin = win.tile([128, n], f32, tag="mwin", name=f"mw{w}")
                nc.vector.tensor_mul(mwin[:], zpad[:, q0 : q0 + n], dwin[:])
                owin = win.tile([128, n], f32, tag="owin", name=f"ow{w}")
                nc.vector.tensor_add(owin[:], hpad[:, q0 : q0 + n], mwin[:])
                o3 = owin.rearrange("p (r c) -> p r c", c=Wp)
                y0 = ROWS_PER_WIN * w
                nc.sync.dma_start(
                    out=out3[:, y0 : y0 + rows, :], in_=o3[:, :rows, 1 : W + 1]
                )

    nc.compile()
    return nc


def _fq_int(w):
    w = np.asarray(w, np.float32)
    scale = (
        np.maximum(np.max(np.abs(w)), np.float32(1e-8)) / np.float32(127.0)
    ).astype(np.float32)
    q = np.clip(np.round(w / scale), -128, 127).astype(np.float32)
    return q, scale


def _prep_gate(wdg, bdg, wpg, bpg):
    qd, sd = _fq_int(wdg)  # [C,1,3,3]
    qp, sp = _fq_int(wpg)  # [HID,C,1,1]
    qp2 = qp[:, :, 0, 0]  # [HID, C]
    # M[t] = (Wp . diag(Wd_t)) scaled into fp8 range: [9, HID, C]
    M = np.empty((9, HID, C), np.float32)
    for ky in range(3):
        for kx in range(3):
            M[3 * ky + kx] = qp2 * qd[:, 0, ky, kx][None, :] / DIV
    # AB plane pack: [128 part, 9 tap, 2 plane, 128 out]
    ab = np.empty((128, 9, 2, 128), np.float32)
    ab[:, :, 0, :] = M[:, :, 0:128].transpose(2, 0, 1)  # h chunk
    ab[:, :, 1, :] = M[:, :, 128:256].transpose(2, 0, 1)  # x0 chunk
    cdw = np.empty((96, 9, 2, 128), np.float32)
    cdw[:, :, 0, :] = M[:, :, 256:352].transpose(2, 0, 1)  # x1[0:96]
    cdw[0:32, :, 1, :] = M[:, :, 352:384].transpose(2, 0, 1)  # x1[96:128]
    cdw[32:96, :, 1, :] = M[:, :, 384:448].transpose(2, 0, 1)  # x2
    scale = np.float32(sd) * np.float32(sp) * DIV
    bias = (
        np.float32(sp) * (qp2 @ np.asarray(bdg, np.float32))
        + np.asarray(bpg, np.float32)
    ).astype(np.float32)
    return (
        np.ascontiguousarray(ab.reshape(128, 9 * 256)).astype(FP8),
        np.ascontiguousarray(cdw.reshape(96, 9 * 256)).astype(FP8),
        scale,
        bias,
    )


def last_in_maps(inputs):
    h = np.asarray(inputs["h"], np.float32)
    x = np.asarray(inputs["x"], np.float32)

    sbt = np.empty((HID, 6), np.float32)
    wmaps = {}
    for i, g in enumerate(("z", "r", "q")):
        ab, cdw, s, b = _prep_gate(
            inputs[f"wd{g}"], inputs[f"bd{g}"], inputs[f"wp{g}"], inputs[f"bp{g}"]
        )
        wmaps[f"wab{g}"] = ab
        wmaps[f"wcd{g}"] = cdw
        sbt[:, 2 * i] = s
        sbt[:, 2 * i + 1] = b

    in_maps = []
    for i in range(B):
        m = {
            "h32": np.ascontiguousarray(h[i].reshape(HID, NPIX)),
            "x32": np.ascontiguousarray(x[i].reshape(INP, NPIX)),
            "sbt": sbt,
        }
        m.update(wmaps)
        in_maps.append(m)
    return in_maps


def kernel(**inputs):
    from concourse.bass_utils import run_bass_kernel_spmd

    if "nc" not in _CACHE:
        _CACHE["nc"] = _build()
    nc = _CACHE["nc"]

    in_maps = last_in_maps(inputs)

    res = run_bass_kernel_spmd(nc, in_maps, list(range(B)))
    out = np.stack(
        [res.results[i]["out"].reshape(HID, H, W) for i in range(B)], axis=0
    )
    return out.astype(np.float32)
